# revision 1
# baseline (speedup 1.0000x reference)
"""Trainium2 Bass kernel for nn_ConvBranch: strided-conv front end + 4 Mamba
layers + final LN + x4 upsample.

Sharding (8 cores): core c = (batch b = c//2, d_inner half j = c%2).
Each core: its batch, full sequence T=2048 (post-conv), full d_model=512,
its 512-channel half of d_inner=1024.  Contractions over d_inner (x_proj,
out_proj) produce partial sums -> pair AllReduce ([0,1],[2,3],[4,5],[6,7]).

Layout: d-major everywhere (features on partitions, time on free dim).
Selective scan: per (d_state n, d-tile g) tensor_tensor_scan along time;
a = exp(A[d,n]*dt) built on ACT with per-partition scale; B/C rows
broadcast across 128 partitions with one-hot K=16 PE matmuls.

Time is processed in TC=512 chunks (SBUF per-partition budget); scan state
chains across chunks via per-(g,n) final-state columns fed to the next
chunk's tensor_tensor_scan initial value.

kernel(**inputs) takes the FULL unsharded inputs, returns (4, 8192, 512).
"""

import sys

import numpy as np

sys.path.insert(0, "/opt/trn_rl_repo")

B_ = 4
D_IN = 256
D = 512          # d_model
STRIDE = 4
KF = 8           # front conv kernel
DS = 16          # d_state
DCONV = 4
DI = 1024        # d_inner
DH = DI // 2     # per-core d_inner half
R = 32           # dt_rank
LN_EPS = 1e-5
P = 128
G = DH // P      # 4
FT = D // P      # 4
XP = 64          # x_proj rows: [dt 0:32 | B 32:48 | C 48:64]
GROUPS = [[0, 1], [2, 3], [4, 5], [6, 7]]


# ====================================================================== build
def build_program(T=2048, NL=4, TC=512):
    import contextlib

    import concourse.bacc as bacc
    import concourse.bass as bass
    import concourse.mybir as mybir
    from concourse.tile import TileContext

    F32 = mybir.dt.float32
    BF16 = mybir.dt.bfloat16
    AF = mybir.ActivationFunctionType
    OP = mybir.AluOpType

    TC = min(TC, T)
    NC = T // TC
    assert TC <= 512
    T_IN = T * STRIDE

    nc = bacc.Bacc("TRN2", target_bir_lowering=False, debug=False,
                   enable_asserts=False, num_devices=8)

    xcatT = nc.dram_tensor("xcatT", [2 * STRIDE * D_IN, T], BF16, kind="ExternalInput")
    wconv = nc.dram_tensor("wconv", [2 * STRIDE * D_IN, D], BF16, kind="ExternalInput")
    conv_bias = nc.dram_tensor("conv_bias", [D, 1], F32, kind="ExternalInput")
    w_in = nc.dram_tensor("w_in", [NL, D, 2 * DH], BF16, kind="ExternalInput")
    b_in = nc.dram_tensor("b_in", [NL, 2 * DH, 1], F32, kind="ExternalInput")
    dconv_wt = nc.dram_tensor("dconv_wt", [NL, DH, DCONV], F32, kind="ExternalInput")
    dconv_bt = nc.dram_tensor("dconv_bt", [NL, DH, 1], F32, kind="ExternalInput")
    w_xp = nc.dram_tensor("w_xp", [NL, DH, XP], BF16, kind="ExternalInput")
    w_dt = nc.dram_tensor("w_dt", [NL, R, DH], F32, kind="ExternalInput")
    b_dt = nc.dram_tensor("b_dt", [NL, DH, 1], F32, kind="ExternalInput")
    a_cols = nc.dram_tensor("a_cols", [NL, DH, DS], F32, kind="ExternalInput")
    d_par = nc.dram_tensor("d_par", [NL, DH, 1], F32, kind="ExternalInput")
    w_out = nc.dram_tensor("w_out", [NL, DH, D], BF16, kind="ExternalInput")
    fn_wb = nc.dram_tensor("fn_wb", [D, 2], F32, kind="ExternalInput")
    identb = nc.dram_tensor("identb", [P, P], F32, kind="ExternalInput")
    selmat = nc.dram_tensor("selmat", [DS, DS * P], F32, kind="ExternalInput")
    y_out = nc.dram_tensor("y_out", [T_IN, D], F32, kind="ExternalOutput")

    NCK = NL * NC
    cc_dbc_i = [nc.dram_tensor(f"cc_dbc_i{k}", [XP, TC], F32) for k in range(NCK)]
    cc_dbc_o = [nc.dram_tensor(f"cc_dbc_o{k}", [XP, TC], F32) for k in range(NCK)]
    cc_y_i = [nc.dram_tensor(f"cc_y_i{l}", [D, T], F32) for l in range(NL)]
    cc_y_o = [nc.dram_tensor(f"cc_y_o{l}", [D, T], F32) for l in range(NL)]

    with TileContext(nc) as tc, contextlib.ExitStack() as ctx:
        persist = ctx.enter_context(tc.tile_pool(name="persist", bufs=1))
        wpool = ctx.enter_context(tc.tile_pool(name="wpool", bufs=1))
        big = ctx.enter_context(tc.tile_pool(name="big", bufs=1))
        scanp = ctx.enter_context(tc.tile_pool(name="scanp", bufs=2))
        small = ctx.enter_context(tc.tile_pool(name="small", bufs=2))
        psum = ctx.enter_context(tc.tile_pool(name="psum", bufs=4, space="PSUM"))
        psum1 = ctx.enter_context(tc.tile_pool(name="psum1", bufs=2, space="PSUM"))

        ones_row = persist.tile([1, P], F32)
        nc.vector.memset(ones_row, 1.0)
        ones_col = persist.tile([P, 1], F32)
        nc.vector.memset(ones_col, 1.0)
        identb_t = persist.tile([P, P], F32)
        nc.sync.dma_start(identb_t, identb[:, :])
        eps_t = persist.tile([P, 1], F32)
        nc.vector.memset(eps_t, LN_EPS)
        sel = persist.tile([DS, DS * P], F32)
        nc.sync.dma_start(sel, selmat[:, :])

        h = [persist.tile([P, T], F32, name=f"h{f}") for f in range(FT)]

        def ln_chunk(t0, wb_t, hi_prec=False):
            """LayerNorm over partitions (d) for columns [t0, t0+TC)."""
            LDT = F32 if hi_prec else BF16
            sl = slice(t0, t0 + TC)
            p1 = psum1.tile([1, TC], F32, tag="stat", name="stat")
            for f in range(FT):
                nc.tensor.matmul(p1, ones_col, h[f][:, sl],
                                 start=(f == 0), stop=(f == FT - 1))
            p2 = psum1.tile([1, TC], F32, tag="stat", name="stat")
            for f in range(FT):
                q = big.tile([P, TC], F32, tag="hsq", name="hsq", bufs=2)
                nc.scalar.activation(q, h[f][:, sl], AF.Square)
                nc.tensor.matmul(p2, ones_col, q,
                                 start=(f == 0), stop=(f == FT - 1))
            s1 = small.tile([1, TC], F32, tag="s1", name="s1")
            nc.scalar.mul(s1, p1, 1.0 / D)          # mean
            s2 = small.tile([1, TC], F32, tag="s2", name="s2")
            nc.scalar.copy(s2, p2)
            msq = small.tile([1, TC], F32, tag="msq", name="msq")
            nc.scalar.activation(msq, s1, AF.Square)
            nc.vector.scalar_tensor_tensor(s2, s2, 1.0 / D, msq,
                                           op0=OP.mult, op1=OP.subtract)
            nc.scalar.activation(s2, s2, AF.Ln, bias=eps_t[0:1, :], scale=1.0)
            nc.scalar.activation(s2, s2, AF.Exp, scale=-0.5)  # rstd
            mrep = big.tile([P, TC], LDT, tag="mrep", name="mrep")
            rrep = big.tile([P, TC], LDT, tag="rrep", name="rrep")
            for (srow, dstt) in ((s1, mrep), (s2, rrep)):
                pt = psum1.tile([P, TC], F32, tag="rep", name="rep")
                nc.tensor.matmul(pt, ones_row, srow, start=True, stop=True)
                nc.scalar.copy(dstt, pt)
            hn = []
            for f in range(FT):
                t2 = big.tile([P, TC], LDT, tag=f"hn{f}", name=f"hn{f}", bufs=2)
                nc.vector.tensor_tensor(t2, h[f][:, sl], mrep, op=OP.subtract)
                nc.vector.tensor_tensor(t2, t2, rrep, op=OP.mult)
                if wb_t is not None:
                    nc.vector.tensor_scalar(t2, t2, wb_t[f][:, 0:1],
                                            wb_t[f][:, 1:2], OP.mult, OP.add)
                hn.append(t2)
            return hn

        # ------------------------------------------------- front conv + GELU
        with tc.tile_pool(name="convp", bufs=1) as convp, \
             tc.tile_pool(name="convx", bufs=4) as convx:
            K16 = (2 * STRIDE * D_IN) // P
            wconv_t = []
            for k in range(K16):
                wt = convp.tile([P, D], BF16, name=f"wconv{k}")
                nc.sync.dma_start(wt, wconv[k * P:(k + 1) * P, :])
                wconv_t.append(wt)
            cb = []
            for f in range(FT):
                cbf = convp.tile([P, 1], F32, name=f"cb{f}")
                nc.sync.dma_start(cbf, conv_bias[f * P:(f + 1) * P, :])
                cb.append(cbf)
            for c in range(T // TC):
                pts = [psum.tile([P, TC], F32, tag="mm", name="mm")
                       for _ in range(FT)]
                for k in range(K16):
                    xt = convx.tile([P, TC], BF16, tag="xcat", name="xcat")
                    nc.sync.dma_start(xt, xcatT[k * P:(k + 1) * P,
                                                c * TC:(c + 1) * TC])
                    for f in range(FT):
                        nc.tensor.matmul(pts[f], wconv_t[k][:, f * P:(f + 1) * P],
                                         xt, start=(k == 0), stop=(k == K16 - 1))
                for f in range(FT):
                    nc.scalar.activation(h[f][:, c * TC:(c + 1) * TC], pts[f],
                                         AF.Gelu, bias=cb[f], scale=1.0)

        # ---------------------------------------------------------- layers
        for l in range(NL):
            w_in_t = [wpool.tile([P, 2 * DH], BF16, tag=f"w_in{k}",
                                 name=f"w_in{k}") for k in range(FT)]
            for k in range(FT):
                nc.sync.dma_start(w_in_t[k], w_in[l, k * P:(k + 1) * P, :])
            b_in_t = [wpool.tile([P, 1], F32, tag=f"b_in{e}", name=f"b_in{e}")
                      for e in range(2 * DH // P)]
            for e in range(2 * DH // P):
                nc.sync.dma_start(b_in_t[e], b_in[l, e * P:(e + 1) * P, :])
            dcw_t = [wpool.tile([P, DCONV], F32, tag=f"dcw{g}", name=f"dcw{g}")
                     for g in range(G)]
            dcb_t = [wpool.tile([P, 1], F32, tag=f"dcb{g}", name=f"dcb{g}")
                     for g in range(G)]
            w_xp_t = [wpool.tile([P, XP], BF16, tag=f"w_xp{g}", name=f"w_xp{g}")
                      for g in range(G)]
            b_dt_t = [wpool.tile([P, 1], F32, tag=f"b_dt{g}", name=f"b_dt{g}")
                      for g in range(G)]
            ac_t = [wpool.tile([P, DS], F32, tag=f"ac{g}", name=f"ac{g}")
                    for g in range(G)]
            dpar_t = [wpool.tile([P, 1], F32, tag=f"dpar{g}", name=f"dpar{g}")
                      for g in range(G)]
            w_out_t = [wpool.tile([P, D], BF16, tag=f"w_out{g}", name=f"w_out{g}")
                       for g in range(G)]
            for g in range(G):
                s = slice(g * P, (g + 1) * P)
                nc.sync.dma_start(dcw_t[g], dconv_wt[l, s, :])
                nc.sync.dma_start(dcb_t[g], dconv_bt[l, s, :])
                nc.sync.dma_start(w_xp_t[g], w_xp[l, s, :])
                nc.sync.dma_start(b_dt_t[g], b_dt[l, s, :])
                nc.sync.dma_start(ac_t[g], a_cols[l, s, :])
                nc.sync.dma_start(dpar_t[g], d_par[l, s, :])
                nc.sync.dma_start(w_out_t[g], w_out[l, s, :])
            w_dt_t = wpool.tile([R, DH], F32, tag="w_dt", name="w_dt")
            nc.sync.dma_start(w_dt_t, w_dt[l])

            fin = [big.tile([P, DS], F32, tag=f"fin{g}", name=f"fin{g}")
                   for g in range(G)]
            halo = [big.tile([P, DCONV - 1], BF16, tag=f"halo{g}",
                             name=f"halo{g}", bufs=2) for g in range(G)]
            for g in range(G):
                nc.vector.memset(halo[g], 0.0)

            for ci in range(NC):
                t0 = ci * TC
                # ---- LN + in_proj
                hn = ln_chunk(t0, None)
                xraw = [big.tile([P, DCONV - 1 + TC], BF16, tag=f"xraw{g}",
                                 name=f"xraw{g}") for g in range(G)]
                zs = [big.tile([P, TC], BF16, tag=f"zs{g}", name=f"zs{g}")
                      for g in range(G)]
                for g in range(G):
                    nc.vector.tensor_copy(xraw[g][:, 0:DCONV - 1], halo[g])
                for half in range(2):
                    for g in range(G):
                        e0 = half * DH + g * P
                        pt = psum.tile([P, TC], F32, tag="mm", name="mm")
                        for k in range(FT):
                            nc.tensor.matmul(pt, w_in_t[k][:, e0:e0 + P],
                                             hn[k], start=(k == 0),
                                             stop=(k == FT - 1))
                        bia = b_in_t[e0 // P]
                        if half == 0:
                            nc.scalar.activation(xraw[g][:, DCONV - 1:],
                                                 pt, AF.Identity, bias=bia,
                                                 scale=1.0)
                        else:
                            nc.scalar.activation(zs[g], pt, AF.Silu, bias=bia,
                                                 scale=1.0)
                # save halo for next chunk, then dconv + silu
                xs = [big.tile([P, TC], BF16, tag=f"xs{g}", name=f"xs{g}")
                      for g in range(G)]
                nhalo = []
                for g in range(G):
                    hl = big.tile([P, DCONV - 1], BF16, tag=f"halo{g}",
                                  name=f"halo{g}", bufs=2)
                    nc.vector.tensor_copy(hl, xraw[g][:, TC:TC + DCONV - 1])
                    nhalo.append(hl)
                    tmp = small.tile([P, TC], BF16, tag="dctmp", name="dctmp",
                                     bufs=2)
                    nc.vector.tensor_scalar(tmp, xraw[g][:, 0:TC],
                                            dcw_t[g][:, 0:1], None, OP.mult)
                    for j in range(1, DCONV):
                        tmp2 = small.tile([P, TC], BF16, tag="dctmp",
                                          name="dctmp", bufs=2)
                        nc.vector.scalar_tensor_tensor(tmp2, xraw[g][:, j:j + TC],
                                                       dcw_t[g][:, j:j + 1], tmp,
                                                       op0=OP.mult, op1=OP.add)
                        tmp = tmp2
                    nc.scalar.activation(xs[g], tmp, AF.Silu, bias=dcb_t[g],
                                         scale=1.0)
                halo = nhalo
                # ---- x_proj partial + pair AllReduce
                ccidx = l * NC + ci
                pt = psum.tile([XP, TC], F32, tag="mm", name="mm")
                for g in range(G):
                    nc.tensor.matmul(pt, w_xp_t[g], xs[g],
                                     start=(g == 0), stop=(g == G - 1))
                dbc_p = small.tile([XP, TC], F32, tag="dbc_p", name="dbc_p")
                nc.scalar.copy(dbc_p, pt)
                nc.sync.dma_start(cc_dbc_i[ccidx][:, :], dbc_p)
                nc.gpsimd.collective_compute(
                    "AllReduce", OP.add, replica_groups=GROUPS,
                    ins=[cc_dbc_i[ccidx][:, :]], outs=[cc_dbc_o[ccidx][:, :]])
                dbc_dt = small.tile([R, TC], F32, tag="dbc_dt", name="dbc_dt")
                nc.sync.dma_start(dbc_dt, cc_dbc_o[ccidx][0:R, :])
                dbc_B = small.tile([DS, TC], F32, tag="dbc_B", name="dbc_B")
                nc.sync.dma_start(dbc_B, cc_dbc_o[ccidx][R:R + DS, :])
                dbc_C = small.tile([DS, TC], F32, tag="dbc_C", name="dbc_C")
                nc.sync.dma_start(dbc_C, cc_dbc_o[ccidx][R + DS:R + 2 * DS, :])
                # ---- dt = softplus(dtw @ dbc_dt + bias); dtu; yacc init
                dt = [big.tile([P, TC], F32, tag=f"dt{g}", name=f"dt{g}")
                      for g in range(G)]
                dtu = [big.tile([P, TC], BF16, tag=f"dtu{g}", name=f"dtu{g}")
                       for g in range(G)]
                yacc = [big.tile([P, TC], F32, tag=f"yacc{g}", name=f"yacc{g}")
                        for g in range(G)]
                for g in range(G):
                    pt = psum.tile([P, TC], F32, tag="mm", name="mm")
                    nc.tensor.matmul(pt, w_dt_t[:, g * P:(g + 1) * P],
                                     dbc_dt, start=True, stop=True)
                    spe = small.tile([P, TC], F32, tag="spe", name="spe")
                    nc.scalar.activation(spe, pt, AF.Exp, bias=b_dt_t[g],
                                         scale=1.0)
                    nc.scalar.activation(dt[g], spe, AF.Ln, bias=1.0, scale=1.0)
                    nc.vector.tensor_tensor(dtu[g], dt[g], xs[g], op=OP.mult)
                    nc.vector.tensor_scalar(yacc[g], xs[g], dpar_t[g], None,
                                            OP.mult)
                # ---- selective scan over d_state
                for n in range(DS):
                    brep = scanp.tile([P, TC], BF16, tag="brep", name="brep")
                    crep = scanp.tile([P, TC], BF16, tag="crep", name="crep")
                    for (ri, (rsrc, dstt)) in enumerate(((dbc_B, brep),
                                                         (dbc_C, crep))):
                        pt = psum1.tile([P, TC], F32, tag="rep", name="rep")
                        nc.tensor.matmul(pt, sel[:, n * P:(n + 1) * P],
                                         rsrc, start=True, stop=True)
                        if (ri + n) % 2 == 0:
                            nc.scalar.copy(dstt, pt)
                        else:
                            nc.vector.tensor_copy(dstt, pt)
                    for g in range(G):
                        a_t = scanp.tile([P, TC], BF16, tag="sc", name="sc",
                                         bufs=8)
                        nc.scalar.activation(a_t, dt[g], AF.Exp,
                                             scale=ac_t[g][:, n:n + 1])
                        b_t = scanp.tile([P, TC], BF16, tag="sc", name="sc",
                                         bufs=8)
                        nc.vector.tensor_tensor(b_t, dtu[g], brep, op=OP.mult)
                        hsc = scanp.tile([P, TC], BF16, tag="sc", name="sc",
                                         bufs=8)
                        init = 0.0 if ci == 0 else fin[g][:, n:n + 1]
                        nc.vector.tensor_tensor_scan(hsc, a_t, b_t, init,
                                                     op0=OP.mult, op1=OP.add)
                        if ci < NC - 1:
                            nc.scalar.copy(fin[g][:, n:n + 1],
                                           hsc[:, TC - 1:TC])
                        cm = scanp.tile([P, TC], BF16, tag="sc", name="sc",
                                        bufs=8)
                        nc.vector.tensor_tensor(cm, hsc, crep, op=OP.mult)
                        eng = nc.gpsimd if ((n * G + g) % 3 == 0) else nc.vector
                        eng.tensor_tensor(yacc[g], yacc[g], cm, op=OP.add)
                # ---- gating + out_proj partial
                yg = [big.tile([P, TC], BF16, tag=f"dtu{g}", name=f"yg{g}")
                      for g in range(G)]
                for g in range(G):
                    nc.vector.tensor_tensor(yg[g], yacc[g], zs[g], op=OP.mult)
                for f in range(FT):
                    pt = psum.tile([P, TC], F32, tag="mm", name="mm")
                    for g in range(G):
                        nc.tensor.matmul(pt, w_out_t[g][:, f * P:(f + 1) * P],
                                         yg[g], start=(g == 0),
                                         stop=(g == G - 1))
                    ot = small.tile([P, TC], F32, tag="oout", name="oout")
                    if (f + ci) % 2 == 0:
                        nc.scalar.copy(ot, pt)
                    else:
                        nc.vector.tensor_copy(ot, pt)
                    nc.sync.dma_start(
                        cc_y_i[l][f * P:(f + 1) * P, t0:t0 + TC], ot)
            # ---- AllReduce y, residual add
            nc.gpsimd.collective_compute(
                "AllReduce", OP.add, replica_groups=GROUPS,
                ins=[cc_y_i[l][:, :]], outs=[cc_y_o[l][:, :]])
            for f in range(FT):
                for c in range(T // TC):
                    yfull = scanp.tile([P, TC], F32, tag="yfull", name="yfull",
                                       bufs=3)
                    nc.sync.dma_start(yfull,
                                      cc_y_o[l][f * P:(f + 1) * P,
                                                c * TC:(c + 1) * TC])
                    eng = nc.vector if (f + c) % 2 == 0 else nc.gpsimd
                    eng.tensor_tensor(h[f][:, c * TC:(c + 1) * TC],
                                      h[f][:, c * TC:(c + 1) * TC],
                                      yfull, op=OP.add)

        # ------------------------------------------- final LN + transpose out
        fn_t = [persist.tile([P, 2], F32, name=f"fn{f}") for f in range(FT)]
        for f in range(FT):
            nc.sync.dma_start(fn_t[f], fn_wb[f * P:(f + 1) * P, :])
        for ci in range(NC):
            t0 = ci * TC
            hf = ln_chunk(t0, fn_t, hi_prec=True)
            for tb in range(TC // P):
                ht = small.tile([P, D], F32, tag="ht", name="ht")
                for f in range(FT):
                    pt = psum1.tile([P, P], F32, tag="rep", name="rep")
                    nc.tensor.transpose(pt, hf[f][:, tb * P:(tb + 1) * P],
                                        identb_t)
                    nc.scalar.copy(ht[:, f * P:(f + 1) * P], pt)
                tglob = t0 + tb * P
                src = ht[:, :]
                rep_in = bass.AP(tensor=src.tensor, offset=src.offset,
                                 ap=[list(src.ap[0]), [0, STRIDE],
                                     list(src.ap[1])])
                dst = y_out[STRIDE * tglob:STRIDE * (tglob + P), :]
                dst3 = dst.rearrange("(t r) d -> t r d", r=STRIDE)
                nc.sync.dma_start(dst3, rep_in)

    nc.compile()
    return nc


# ================================================================ host side
def make_core_inputs(inputs, T=2048, NL=4):
    x = np.asarray(inputs["x"], np.float32)
    conv_w = np.asarray(inputs["conv_w"], np.float32)
    conv_b = np.asarray(inputs["conv_b"], np.float32)
    in_proj_w = np.asarray(inputs["in_proj_w"], np.float32)
    dconv_w = np.asarray(inputs["dconv_w"], np.float32)
    dconv_b = np.asarray(inputs["dconv_b"], np.float32)
    x_proj_w = np.asarray(inputs["x_proj_w"], np.float32)
    dt_proj_w = np.asarray(inputs["dt_proj_w"], np.float32)
    dt_proj_b = np.asarray(inputs["dt_proj_b"], np.float32)
    A_log = np.asarray(inputs["A_log"], np.float32)
    D_param = np.asarray(inputs["D_param"], np.float32)
    out_proj_w = np.asarray(inputs["out_proj_w"], np.float32)
    ln_w = np.asarray(inputs["ln_w"], np.float32)
    ln_b = np.asarray(inputs["ln_b"], np.float32)
    fn_w = np.asarray(inputs["fn_w"], np.float32)
    fn_b = np.asarray(inputs["fn_b"], np.float32)

    Bn = x.shape[0]
    di = x.shape[2]
    dmodel = conv_w.shape[0]
    dinner = in_proj_w.shape[1] // 2
    dh = dinner // 2

    xpad = np.concatenate([np.zeros((Bn, KF - 1, di), np.float32), x], axis=1)
    idx = np.arange(T)[:, None] * STRIDE + np.arange(KF)[None, :]
    xcat = xpad[:, idx, :].reshape(Bn, T, KF * di)
    xcatT = np.ascontiguousarray(xcat.transpose(0, 2, 1))
    wconv = np.ascontiguousarray(conv_w.transpose(2, 1, 0).reshape(KF * di, dmodel))

    A = -np.exp(A_log)

    per_core = []
    for c in range(8):
        b, j = c // 2, c % 2
        sl = slice(j * dh, (j + 1) * dh)
        w_in_l, b_in_l, w_out_l, w_xp_l = [], [], [], []
        for l in range(NL):
            Wx = in_proj_w[l, :dinner][sl] * ln_w[l][None, :]
            Wz = in_proj_w[l, dinner:][sl] * ln_w[l][None, :]
            w_in_l.append(np.concatenate([Wx.T, Wz.T], axis=1))
            bx = in_proj_w[l, :dinner][sl] @ ln_b[l]
            bz = in_proj_w[l, dinner:][sl] @ ln_b[l]
            b_in_l.append(np.concatenate([bx, bz])[:, None])
            w_out_l.append(out_proj_w[l][:, sl].T)
            w_xp_l.append(np.ascontiguousarray(x_proj_w[l][:, sl].T))
        d = dict(
            xcatT=xcatT[b],
            wconv=wconv,
            conv_bias=conv_b[:, None],
            w_in=np.stack(w_in_l),
            b_in=np.stack(b_in_l),
            dconv_wt=dconv_w[:, sl, :],
            dconv_bt=dconv_b[:, sl, None],
            w_xp=np.stack(w_xp_l),
            w_dt=np.ascontiguousarray(dt_proj_w[:, sl, :].transpose(0, 2, 1)),
            b_dt=dt_proj_b[:, sl, None],
            a_cols=A[:, sl, :],
            d_par=D_param[:, sl, None],
            w_out=np.stack(w_out_l),
            fn_wb=np.stack([fn_w, fn_b], axis=1),
            identb=np.eye(P, dtype=np.float32),
            selmat=np.repeat(np.eye(DS, dtype=np.float32), P, axis=1),
        )
        per_core.append(d)
    return per_core


def cast_core_inputs(nc, per_core):
    import concourse.mybir as mybir
    want = {}
    for alloc in nc.m.functions[0].allocations:
        if getattr(alloc, "kind", None) == "ExternalInput":
            want[alloc.memorylocations[0].name] = mybir.dt.np(alloc.dtype)
    return [{k: np.ascontiguousarray(np.asarray(v).astype(want[k]))
             for k, v in d.items() if k in want} for d in per_core]


_PROGRAM_CACHE = {}


def get_program(T=2048, NL=4, TC=512):
    key = (T, NL, TC)
    if key not in _PROGRAM_CACHE:
        _PROGRAM_CACHE[key] = build_program(T, NL, TC)
    return _PROGRAM_CACHE[key]


def kernel(**inputs):
    from concourse.bass_utils import run_bass_kernel_spmd
    T = inputs["x"].shape[1] // STRIDE
    NL = inputs["in_proj_w"].shape[0]
    nc = get_program(T, NL)
    per_core = cast_core_inputs(nc, make_core_inputs(inputs, T, NL))
    res = run_bass_kernel_spmd(nc, per_core, core_ids=list(range(8)))
    Bn = inputs["x"].shape[0]
    y = np.stack([res.results[2 * b]["y_out"] for b in range(Bn)])
    return y.astype(np.float32)



# revision 9
# speedup vs baseline: 1.2010x; 1.2010x over previous
"""Trainium2 Bass kernel for nn_ConvBranch: strided-conv front end + 4 Mamba
layers + final LN + x4 upsample.

Sharding (8 cores): core c = (batch b = c//2, d_inner half j = c%2).
Each core: its batch, full sequence T=2048 (post-conv), full d_model=512,
its 512-channel half of d_inner=1024.  Contractions over d_inner (x_proj,
out_proj) produce partial sums -> pair AllReduce ([0,1],[2,3],[4,5],[6,7]).

Layout: d-major everywhere (features on partitions, time on free dim).
Selective scan: per (d_state n, d-tile g) tensor_tensor_scan along time;
a = exp(A[d,n]*dt) built on ACT with per-partition scale; B/C rows
broadcast across 128 partitions with one-hot K=16 PE matmuls.

v2: software-pipelined chunk stages (prescan of stage s+1 emitted before
scan phase of stage s) to hide the dbc AllReduce; per-chunk bf16
y AllReduce; y accumulation over d_state moved to PE (identity-matmul
accumulate into PSUM); fin state copies + part of dconv/cm on GpSimd.
"""

import sys

import numpy as np

sys.path.insert(0, "/opt/trn_rl_repo")

B_ = 4
D_IN = 256
D = 512          # d_model
STRIDE = 4
KF = 8           # front conv kernel
DS = 16          # d_state
DCONV = 4
DI = 1024        # d_inner
DH = DI // 2     # per-core d_inner half
P = 128
G = DH // P      # 4
FT = D // P      # 4
XP = 64          # x_proj rows: [dt 0:32 | B 32:48 | C 48:64]
R = 32           # dt_rank
LN_EPS = 1e-5
GROUPS = [[0, 1], [2, 3], [4, 5], [6, 7]]


# ====================================================================== build
def build_program(T=2048, NL=4, TC=512):
    import contextlib

    import concourse.bacc as bacc
    import concourse.bass as bass
    import concourse.mybir as mybir
    from concourse.tile import TileContext

    F32 = mybir.dt.float32
    BF16 = mybir.dt.bfloat16
    AF = mybir.ActivationFunctionType
    OP = mybir.AluOpType

    TC = min(TC, T)
    NC = T // TC
    assert TC <= 512
    T_IN = T * STRIDE

    nc = bacc.Bacc("TRN2", target_bir_lowering=False, debug=False,
                   enable_asserts=False, num_devices=8)

    xcatT = nc.dram_tensor("xcatT", [2 * STRIDE * D_IN, T], BF16, kind="ExternalInput")
    wconv = nc.dram_tensor("wconv", [2 * STRIDE * D_IN, D], BF16, kind="ExternalInput")
    conv_bias = nc.dram_tensor("conv_bias", [D, 1], F32, kind="ExternalInput")
    w_in = nc.dram_tensor("w_in", [NL, D, 2 * DH], BF16, kind="ExternalInput")
    b_in = nc.dram_tensor("b_in", [NL, 2 * DH, 1], F32, kind="ExternalInput")
    dconv_wt = nc.dram_tensor("dconv_wt", [NL, DH, DCONV], F32, kind="ExternalInput")
    dconv_bt = nc.dram_tensor("dconv_bt", [NL, DH, 1], F32, kind="ExternalInput")
    w_xp = nc.dram_tensor("w_xp", [NL, DH, XP], BF16, kind="ExternalInput")
    w_dt = nc.dram_tensor("w_dt", [NL, R, DH], F32, kind="ExternalInput")
    b_dt = nc.dram_tensor("b_dt", [NL, DH, 1], F32, kind="ExternalInput")
    a_cols = nc.dram_tensor("a_cols", [NL, DH, DS], F32, kind="ExternalInput")
    d_par = nc.dram_tensor("d_par", [NL, DH, 1], F32, kind="ExternalInput")
    w_out = nc.dram_tensor("w_out", [NL, DH, D], BF16, kind="ExternalInput")
    fn_wb = nc.dram_tensor("fn_wb", [D, 2], F32, kind="ExternalInput")
    identb = nc.dram_tensor("identb", [P, P], F32, kind="ExternalInput")
    selmat = nc.dram_tensor("selmat", [DS, DS * P], F32, kind="ExternalInput")
    y_out = nc.dram_tensor("y_out", [T_IN, D], F32, kind="ExternalOutput")

    NCK = NL * NC
    cc_dbc_i = [nc.dram_tensor(f"cc_dbc_i{k}", [XP, TC], F32) for k in range(NCK)]
    cc_dbc_o = [nc.dram_tensor(f"cc_dbc_o{k}", [XP, TC], F32) for k in range(NCK)]
    cc_y_i = [nc.dram_tensor(f"cc_y_i{k}", [D, TC], BF16) for k in range(NCK)]
    cc_y_o = [nc.dram_tensor(f"cc_y_o{k}", [D, TC], BF16) for k in range(NCK)]

    with TileContext(nc) as tc, contextlib.ExitStack() as ctx:
        persist = ctx.enter_context(tc.tile_pool(name="persist", bufs=1))
        wpool = ctx.enter_context(tc.tile_pool(name="wpool", bufs=2))
        big = ctx.enter_context(tc.tile_pool(name="big", bufs=1))
        scanp = ctx.enter_context(tc.tile_pool(name="scanp", bufs=2))
        small = ctx.enter_context(tc.tile_pool(name="small", bufs=2))
        psum = ctx.enter_context(tc.tile_pool(name="psum", bufs=2, space="PSUM"))
        psum1 = ctx.enter_context(tc.tile_pool(name="psum1", bufs=2, space="PSUM"))
        psumy = ctx.enter_context(tc.tile_pool(name="psumy", bufs=1, space="PSUM"))

        ones_row = persist.tile([1, P], F32)
        nc.vector.memset(ones_row, 1.0)
        ones_col = persist.tile([P, 1], F32)
        nc.vector.memset(ones_col, 1.0)
        identb_t = persist.tile([P, P], F32)
        nc.sync.dma_start(identb_t, identb[:, :])
        ident_bf = persist.tile([P, P], BF16)
        nc.scalar.copy(ident_bf, identb_t)
        eps_t = persist.tile([P, 1], F32)
        nc.vector.memset(eps_t, LN_EPS)
        sel = persist.tile([DS, DS * P], F32)
        nc.sync.dma_start(sel, selmat[:, :])

        h = [persist.tile([P, T], F32, name=f"h{f}") for f in range(FT)]

        def ln_chunk(t0, wb_t, hi_prec=False):
            """LayerNorm over partitions (d) for columns [t0, t0+TC)."""
            LDT = F32 if hi_prec else BF16
            sl = slice(t0, t0 + TC)
            p1 = psum1.tile([1, TC], F32, tag="rep", name="stat1")
            for f in range(FT):
                nc.tensor.matmul(p1, ones_col, h[f][:, sl],
                                 start=(f == 0), stop=(f == FT - 1))
            p2 = psum1.tile([1, TC], F32, tag="rep", name="stat2")
            for f in range(FT):
                q = big.tile([P, TC], F32, tag="hsq", name="hsq", bufs=2)
                nc.scalar.activation(q, h[f][:, sl], AF.Square)
                nc.tensor.matmul(p2, ones_col, q,
                                 start=(f == 0), stop=(f == FT - 1))
            s1 = small.tile([1, TC], F32, tag="s1", name="s1")
            nc.scalar.mul(s1, p1, 1.0 / D)          # mean
            s2 = small.tile([1, TC], F32, tag="s2", name="s2")
            nc.scalar.copy(s2, p2)
            msq = small.tile([1, TC], F32, tag="msq", name="msq")
            nc.scalar.activation(msq, s1, AF.Square)
            nc.vector.scalar_tensor_tensor(s2, s2, 1.0 / D, msq,
                                           op0=OP.mult, op1=OP.subtract)
            nc.scalar.activation(s2, s2, AF.Ln, bias=eps_t[0:1, :], scale=1.0)
            nc.scalar.activation(s2, s2, AF.Exp, scale=-0.5)  # rstd
            mrep = big.tile([P, TC], LDT, tag="mrep", name="mrep")
            rrep = big.tile([P, TC], LDT, tag="rrep", name="rrep")
            for (srow, dstt) in ((s1, mrep), (s2, rrep)):
                pt = psum1.tile([P, TC], F32, tag="rep", name="rep")
                nc.tensor.matmul(pt, ones_row, srow, start=True, stop=True)
                nc.scalar.copy(dstt, pt)
            hn = []
            for f in range(FT):
                t2 = big.tile([P, TC], LDT, tag=f"hn{f}", name=f"hn{f}", bufs=2)
                nc.vector.tensor_tensor(t2, h[f][:, sl], mrep, op=OP.subtract)
                nc.vector.tensor_tensor(t2, t2, rrep, op=OP.mult)
                if wb_t is not None:
                    nc.vector.tensor_scalar(t2, t2, wb_t[f][:, 0:1],
                                            wb_t[f][:, 1:2], OP.mult, OP.add)
                hn.append(t2)
            return hn

        # ------------------------------------------------- front conv + GELU
        with tc.tile_pool(name="convp", bufs=1) as convp, \
             tc.tile_pool(name="convx", bufs=4) as convx:
            K16 = (2 * STRIDE * D_IN) // P
            cb = []
            for f in range(FT):
                cbf = convp.tile([P, 1], F32, name=f"cb{f}")
                nc.sync.dma_start(cbf, conv_bias[f * P:(f + 1) * P, :])
                cb.append(cbf)
            for c in range(T // TC):
                pts = [psumy.tile([P, TC], F32, tag=f"py{f}", name="cmm")
                       for f in range(FT)]
                for k in range(K16):
                    wt = convp.tile([P, D], BF16, tag="wc", name="wc", bufs=4)
                    nc.sync.dma_start(wt, wconv[k * P:(k + 1) * P, :])
                    xt = convx.tile([P, TC], BF16, tag="xcat", name="xcat")
                    nc.sync.dma_start(xt, xcatT[k * P:(k + 1) * P,
                                                c * TC:(c + 1) * TC])
                    for f in range(FT):
                        nc.tensor.matmul(pts[f], wt[:, f * P:(f + 1) * P],
                                         xt, start=(k == 0), stop=(k == K16 - 1))
                for f in range(FT):
                    nc.scalar.activation(h[f][:, c * TC:(c + 1) * TC], pts[f],
                                         AF.Gelu, bias=cb[f], scale=1.0)

        # ---------------------------------------------------------- layers
        # Per-layer weight tiles (wpool bufs=2 rotates across layers).
        def load_weights(l):
            w_in_t = [wpool.tile([P, 2 * DH], BF16, tag=f"w_in{k}",
                                 name=f"w_in{k}") for k in range(FT)]
            for k in range(FT):
                nc.sync.dma_start(w_in_t[k], w_in[l, k * P:(k + 1) * P, :])
            b_in_t = [wpool.tile([P, 1], F32, tag=f"b_in{e}", name=f"b_in{e}")
                      for e in range(2 * DH // P)]
            for e in range(2 * DH // P):
                nc.sync.dma_start(b_in_t[e], b_in[l, e * P:(e + 1) * P, :])
            dcw_t = [wpool.tile([P, DCONV], F32, tag=f"dcw{g}", name=f"dcw{g}")
                     for g in range(G)]
            dcb_t = [wpool.tile([P, 1], F32, tag=f"dcb{g}", name=f"dcb{g}")
                     for g in range(G)]
            w_xp_t = [wpool.tile([P, XP], BF16, tag=f"w_xp{g}", name=f"w_xp{g}")
                      for g in range(G)]
            b_dt_t = [wpool.tile([P, 1], F32, tag=f"b_dt{g}", name=f"b_dt{g}")
                      for g in range(G)]
            ac_t = [wpool.tile([P, DS], F32, tag=f"ac{g}", name=f"ac{g}")
                    for g in range(G)]
            dpar_t = [wpool.tile([P, 1], F32, tag=f"dpar{g}", name=f"dpar{g}")
                      for g in range(G)]
            w_out_t = [wpool.tile([P, D], BF16, tag=f"w_out{g}", name=f"w_out{g}")
                       for g in range(G)]
            for g in range(G):
                s = slice(g * P, (g + 1) * P)
                nc.sync.dma_start(dcw_t[g], dconv_wt[l, s, :])
                nc.sync.dma_start(dcb_t[g], dconv_bt[l, s, :])
                nc.sync.dma_start(w_xp_t[g], w_xp[l, s, :])
                nc.sync.dma_start(b_dt_t[g], b_dt[l, s, :])
                nc.sync.dma_start(ac_t[g], a_cols[l, s, :])
                nc.sync.dma_start(dpar_t[g], d_par[l, s, :])
                nc.sync.dma_start(w_out_t[g], w_out[l, s, :])
            w_dt_t = wpool.tile([R, DH], F32, tag="w_dt", name="w_dt")
            nc.sync.dma_start(w_dt_t, w_dt[l])
            dg_t = []
            for g in range(G):
                dgg = []
                for j in range(DCONV):
                    dg = wpool.tile([P, P], BF16, tag=f"dg{g}_{j}",
                                    name=f"dg{g}_{j}", bufs=1)
                    nc.vector.tensor_scalar(dg, ident_bf, dcw_t[g][:, j:j + 1],
                                            None, OP.mult)
                    dgg.append(dg)
                dg_t.append(dgg)
            return dict(w_in=w_in_t, b_in=b_in_t, dg=dg_t, dcb=dcb_t,
                        w_xp=w_xp_t, b_dt=b_dt_t, ac=ac_t, dpar=dpar_t,
                        w_out=w_out_t, w_dt=w_dt_t)

        layer_state = {}   # l -> dict(halo=[g tiles])

        def prescan(l, ci, W):
            """LN + in_proj + dconv + x_proj + AllReduce + dt for chunk ci."""
            t0 = ci * TC
            if ci == 0:
                halo = [big.tile([P, DCONV - 1], BF16, tag=f"halo{g}",
                                 name=f"halo{g}", bufs=2) for g in range(G)]
                for g in range(G):
                    nc.vector.memset(halo[g], 0.0)
                layer_state[l] = dict(halo=halo)
            halo = layer_state[l]["halo"]
            hn = ln_chunk(t0, None)
            xraw = [big.tile([P, DCONV - 1 + TC], BF16, tag=f"xraw{g}",
                             name=f"xraw{g}", bufs=2) for g in range(G)]
            zs = [big.tile([P, TC], BF16, tag=f"zs{g}", name=f"zs{g}", bufs=2)
                  for g in range(G)]
            for g in range(G):
                nc.vector.tensor_copy(xraw[g][:, 0:DCONV - 1], halo[g])
            for half in range(2):
                for g in range(G):
                    e0 = half * DH + g * P
                    pt = psum.tile([P, TC], F32, tag="mm", name="mm")
                    for k in range(FT):
                        nc.tensor.matmul(pt, W["w_in"][k][:, e0:e0 + P],
                                         hn[k], start=(k == 0),
                                         stop=(k == FT - 1))
                    bia = W["b_in"][e0 // P]
                    if half == 0:
                        nc.scalar.activation(xraw[g][:, DCONV - 1:],
                                             pt, AF.Identity, bias=bia,
                                             scale=1.0)
                    else:
                        nc.scalar.activation(zs[g], pt, AF.Silu, bias=bia,
                                             scale=1.0)
            # save halo for next chunk, then dconv (DVE for g 0,2; Pool 1,3)
            xs = [big.tile([P, TC], BF16, tag=f"xs{g}", name=f"xs{g}", bufs=2)
                  for g in range(G)]
            nhalo = []
            for g in range(G):
                hl = big.tile([P, DCONV - 1], BF16, tag=f"halo{g}",
                              name=f"halo{g}", bufs=2)
                nc.vector.tensor_copy(hl, xraw[g][:, TC:TC + DCONV - 1])
                nhalo.append(hl)
                pt = psum.tile([P, TC], F32, tag="mm", name="mm")
                for j in range(DCONV):
                    nc.tensor.matmul(pt, W["dg"][g][j], xraw[g][:, j:j + TC],
                                     start=(j == 0), stop=(j == DCONV - 1))
                nc.scalar.activation(xs[g], pt, AF.Silu, bias=W["dcb"][g],
                                     scale=1.0)
            layer_state[l]["halo"] = nhalo
            # ---- x_proj partial + pair AllReduce
            ccidx = l * NC + ci
            pt = psum.tile([XP, TC], F32, tag="mm", name="mm")
            for g in range(G):
                nc.tensor.matmul(pt, W["w_xp"][g], xs[g],
                                 start=(g == 0), stop=(g == G - 1))
            dbc_p = small.tile([XP, TC], F32, tag="dbc_p", name="dbc_p")
            nc.scalar.copy(dbc_p, pt)
            nc.sync.dma_start(cc_dbc_i[ccidx][:, :], dbc_p)
            nc.gpsimd.collective_compute(
                "AllReduce", OP.add, replica_groups=GROUPS,
                ins=[cc_dbc_i[ccidx][:, :]], outs=[cc_dbc_o[ccidx][:, :]])
            return dict(xs=xs, zs=zs, W=W)

        def build_rep(n, dbc_B, dbc_C):
            """Broadcast B/C row n across partitions via one-hot PE matmul."""
            brep = scanp.tile([P, TC], BF16, tag="brep", name="brep", bufs=2)
            crep = scanp.tile([P, TC], BF16, tag="crep", name="crep", bufs=2)
            for ri, (rsrc, dstt) in enumerate(((dbc_B, brep), (dbc_C, crep))):
                pt = psum1.tile([P, TC], F32, tag="rep", name="rep")
                nc.tensor.matmul(pt, sel[:, n * P:(n + 1) * P],
                                 rsrc, start=True, stop=True)
                if (ri + n) % 2 == 0:
                    nc.scalar.copy(dstt, pt)
                else:
                    nc.vector.tensor_copy(dstt, pt)
            return brep, crep

        def scanphase(l, ci, S):
            """dt path + selective scan + gating + out_proj + AR + residual."""
            t0 = ci * TC
            ccidx = l * NC + ci
            W = S["W"]
            xs, zs = S["xs"], S["zs"]
            if ci == 0:
                fin = [big.tile([P, DS], F32, tag=f"fin{g}", name=f"fin{g}")
                       for g in range(G)]
                layer_state[l]["fin"] = fin
            fin = layer_state[l]["fin"]
            # dbc split (AllReduce has had a full stage to finish)
            dbc_dt = small.tile([R, TC], F32, tag="dbc_dt", name="dbc_dt")
            nc.sync.dma_start(dbc_dt, cc_dbc_o[ccidx][0:R, :])
            dbc_B = small.tile([DS, TC], F32, tag="dbc_B", name="dbc_B")
            nc.sync.dma_start(dbc_B, cc_dbc_o[ccidx][R:R + DS, :])
            dbc_C = small.tile([DS, TC], F32, tag="dbc_C", name="dbc_C")
            nc.sync.dma_start(dbc_C, cc_dbc_o[ccidx][R + DS:R + 2 * DS, :])
            # dt = softplus(dtw @ dbc_dt + bias); dtu; dxs
            dt = [big.tile([P, TC], F32, tag=f"dt{g}", name=f"dt{g}")
                  for g in range(G)]
            dtu = [big.tile([P, TC], BF16, tag=f"dtu{g}", name=f"dtu{g}")
                   for g in range(G)]
            dxs = [big.tile([P, TC], BF16, tag=f"dxs{g}", name=f"dxs{g}")
                   for g in range(G)]
            for g in range(G):
                pt = psum.tile([P, TC], F32, tag="mm", name="mm")
                nc.tensor.matmul(pt, W["w_dt"][:, g * P:(g + 1) * P],
                                 dbc_dt, start=True, stop=True)
                spe = small.tile([P, TC], F32, tag="spe", name="spe")
                nc.scalar.activation(spe, pt, AF.Exp, bias=W["b_dt"][g],
                                     scale=1.0)
                nc.scalar.activation(dt[g], spe, AF.Ln, bias=1.0, scale=1.0)
                nc.vector.tensor_tensor(dtu[g], dt[g], xs[g], op=OP.mult)
                nc.vector.tensor_scalar(dxs[g], xs[g], W["dpar"][g], None,
                                        OP.mult)
            psy = [psumy.tile([P, TC], F32, tag=f"py{g}", name=f"py{g}")
                   for g in range(G)]
            for g in range(G):
                nc.tensor.matmul(psy[g], ident_bf, dxs[g],
                                 start=True, stop=False)
            reps = build_rep(0, dbc_B, dbc_C)
            for n in range(DS):
                brep, crep = reps
                if n + 1 < DS:
                    reps = build_rep(n + 1, dbc_B, dbc_C)
                for g in range(G):
                    a_t = scanp.tile([P, TC], BF16, tag="sc", name="sc",
                                     bufs=8)
                    nc.scalar.activation(a_t, dt[g], AF.Exp,
                                         scale=W["ac"][g][:, n:n + 1])
                    b_t = scanp.tile([P, TC], BF16, tag="sc", name="sc",
                                     bufs=8)
                    nc.vector.tensor_tensor(b_t, dtu[g], brep, op=OP.mult)
                    hsc = scanp.tile([P, TC], BF16, tag="sc", name="sc",
                                     bufs=8)
                    init = 0.0 if ci == 0 else fin[g][:, n:n + 1]
                    nc.vector.tensor_tensor_scan(hsc, a_t, b_t, init,
                                                 op0=OP.mult, op1=OP.add)
                    if ci < NC - 1:
                        if (n + g) % 2 == 0:
                            nc.scalar.copy(fin[g][:, n:n + 1],
                                           hsc[:, TC - 1:TC])
                        else:
                            nc.vector.tensor_copy(fin[g][:, n:n + 1],
                                                  hsc[:, TC - 1:TC])
                    cm = scanp.tile([P, TC], BF16, tag="sc", name="sc",
                                    bufs=8)
                    nc.gpsimd.tensor_tensor(cm, hsc, crep, op=OP.mult)
                    nc.tensor.matmul(psy[g], ident_bf, cm,
                                     start=False, stop=(n == DS - 1))
            # ---- gating + out_proj partial (bf16 collective)
            yg = [big.tile([P, TC], BF16, tag=f"dtu{g}", name=f"yg{g}")
                  for g in range(G)]
            for g in range(G):
                nc.vector.tensor_tensor(yg[g], psy[g], zs[g], op=OP.mult)
            for f in range(FT):
                pt = psum.tile([P, TC], F32, tag="mm", name="mm")
                for g in range(G):
                    nc.tensor.matmul(pt, W["w_out"][g][:, f * P:(f + 1) * P],
                                     yg[g], start=(g == 0),
                                     stop=(g == G - 1))
                ot = small.tile([P, TC], BF16, tag="oout", name="oout")
                if (f + ci) % 2 == 0:
                    nc.scalar.copy(ot, pt)
                else:
                    nc.vector.tensor_copy(ot, pt)
                nc.sync.dma_start(cc_y_i[ccidx][f * P:(f + 1) * P, :], ot)
            nc.gpsimd.collective_compute(
                "AllReduce", OP.add, replica_groups=GROUPS,
                ins=[cc_y_i[ccidx][:, :]], outs=[cc_y_o[ccidx][:, :]])
            for f in range(FT):
                yfull = scanp.tile([P, TC], BF16, tag="yfull", name="yfull",
                                   bufs=3)
                nc.sync.dma_start(yfull, cc_y_o[ccidx][f * P:(f + 1) * P, :])
                eng = nc.vector if (f + ci) % 2 == 0 else nc.gpsimd
                eng.tensor_tensor(h[f][:, t0:t0 + TC],
                                  h[f][:, t0:t0 + TC], yfull, op=OP.add)

        # Flat software pipeline over (layer, chunk) stages.
        stages = [(l, ci) for l in range(NL) for ci in range(NC)]
        Ws = {}
        Ws[0] = load_weights(0)
        pre = {}
        pre[stages[0]] = prescan(0, 0, Ws[0])
        for si, (l, ci) in enumerate(stages):
            if si + 1 < len(stages):
                l2, ci2 = stages[si + 1]
                if ci2 == 0:
                    Ws[l2] = load_weights(l2)
                pre[stages[si + 1]] = prescan(l2, ci2, Ws[l2])
            scanphase(l, ci, pre.pop((l, ci)))

        # ------------------------------------------- final LN + transpose out
        fn_t = [persist.tile([P, 2], F32, name=f"fn{f}") for f in range(FT)]
        for f in range(FT):
            nc.sync.dma_start(fn_t[f], fn_wb[f * P:(f + 1) * P, :])
        for ci in range(NC):
            t0 = ci * TC
            hf = ln_chunk(t0, fn_t, hi_prec=True)
            for tb in range(TC // P):
                ht = small.tile([P, D], F32, tag="ht", name="ht")
                for f in range(FT):
                    pt = psum1.tile([P, P], F32, tag="rep", name="rep")
                    nc.tensor.transpose(pt, hf[f][:, tb * P:(tb + 1) * P],
                                        identb_t)
                    nc.scalar.copy(ht[:, f * P:(f + 1) * P], pt)
                tglob = t0 + tb * P
                src = ht[:, :]
                rep_in = bass.AP(tensor=src.tensor, offset=src.offset,
                                 ap=[list(src.ap[0]), [0, STRIDE],
                                     list(src.ap[1])])
                dst = y_out[STRIDE * tglob:STRIDE * (tglob + P), :]
                dst3 = dst.rearrange("(t r) d -> t r d", r=STRIDE)
                nc.sync.dma_start(dst3, rep_in)

    nc.compile()
    return nc


# ================================================================ host side
def make_core_inputs(inputs, T=2048, NL=4):
    x = np.asarray(inputs["x"], np.float32)
    conv_w = np.asarray(inputs["conv_w"], np.float32)
    conv_b = np.asarray(inputs["conv_b"], np.float32)
    in_proj_w = np.asarray(inputs["in_proj_w"], np.float32)
    dconv_w = np.asarray(inputs["dconv_w"], np.float32)
    dconv_b = np.asarray(inputs["dconv_b"], np.float32)
    x_proj_w = np.asarray(inputs["x_proj_w"], np.float32)
    dt_proj_w = np.asarray(inputs["dt_proj_w"], np.float32)
    dt_proj_b = np.asarray(inputs["dt_proj_b"], np.float32)
    A_log = np.asarray(inputs["A_log"], np.float32)
    D_param = np.asarray(inputs["D_param"], np.float32)
    out_proj_w = np.asarray(inputs["out_proj_w"], np.float32)
    ln_w = np.asarray(inputs["ln_w"], np.float32)
    ln_b = np.asarray(inputs["ln_b"], np.float32)
    fn_w = np.asarray(inputs["fn_w"], np.float32)
    fn_b = np.asarray(inputs["fn_b"], np.float32)

    Bn = x.shape[0]
    di = x.shape[2]
    dmodel = conv_w.shape[0]
    dinner = in_proj_w.shape[1] // 2
    dh = dinner // 2

    xpad = np.concatenate([np.zeros((Bn, KF - 1, di), np.float32), x], axis=1)
    idx = np.arange(T)[:, None] * STRIDE + np.arange(KF)[None, :]
    xcat = xpad[:, idx, :].reshape(Bn, T, KF * di)
    xcatT = np.ascontiguousarray(xcat.transpose(0, 2, 1))
    wconv = np.ascontiguousarray(conv_w.transpose(2, 1, 0).reshape(KF * di, dmodel))

    A = -np.exp(A_log)

    per_core = []
    for c in range(8):
        b, j = c // 2, c % 2
        sl = slice(j * dh, (j + 1) * dh)
        w_in_l, b_in_l, w_out_l, w_xp_l = [], [], [], []
        for l in range(NL):
            Wx = in_proj_w[l, :dinner][sl] * ln_w[l][None, :]
            Wz = in_proj_w[l, dinner:][sl] * ln_w[l][None, :]
            w_in_l.append(np.concatenate([Wx.T, Wz.T], axis=1))
            bx = in_proj_w[l, :dinner][sl] @ ln_b[l]
            bz = in_proj_w[l, dinner:][sl] @ ln_b[l]
            b_in_l.append(np.concatenate([bx, bz])[:, None])
            w_out_l.append(out_proj_w[l][:, sl].T)
            w_xp_l.append(np.ascontiguousarray(x_proj_w[l][:, sl].T))
        d = dict(
            xcatT=xcatT[b],
            wconv=wconv,
            conv_bias=conv_b[:, None],
            w_in=np.stack(w_in_l),
            b_in=np.stack(b_in_l),
            dconv_wt=dconv_w[:, sl, :],
            dconv_bt=dconv_b[:, sl, None],
            w_xp=np.stack(w_xp_l),
            w_dt=np.ascontiguousarray(dt_proj_w[:, sl, :].transpose(0, 2, 1)),
            b_dt=dt_proj_b[:, sl, None],
            a_cols=A[:, sl, :],
            d_par=D_param[:, sl, None],
            w_out=np.stack(w_out_l),
            fn_wb=np.stack([fn_w, fn_b], axis=1),
            identb=np.eye(P, dtype=np.float32),
            selmat=np.repeat(np.eye(DS, dtype=np.float32), P, axis=1),
        )
        per_core.append(d)
    return per_core


def cast_core_inputs(nc, per_core):
    import concourse.mybir as mybir
    want = {}
    for alloc in nc.m.functions[0].allocations:
        if getattr(alloc, "kind", None) == "ExternalInput":
            want[alloc.memorylocations[0].name] = mybir.dt.np(alloc.dtype)
    return [{k: np.ascontiguousarray(np.asarray(v).astype(want[k]))
             for k, v in d.items() if k in want} for d in per_core]


_PROGRAM_CACHE = {}


def get_program(T=2048, NL=4, TC=512):
    key = (T, NL, TC)
    if key not in _PROGRAM_CACHE:
        _PROGRAM_CACHE[key] = build_program(T, NL, TC)
    return _PROGRAM_CACHE[key]


def kernel(**inputs):
    from concourse.bass_utils import run_bass_kernel_spmd
    T = inputs["x"].shape[1] // STRIDE
    NL = inputs["in_proj_w"].shape[0]
    nc = get_program(T, NL)
    per_core = cast_core_inputs(nc, make_core_inputs(inputs, T, NL))
    res = run_bass_kernel_spmd(nc, per_core, core_ids=list(range(8)))
    Bn = inputs["x"].shape[0]
    y = np.stack([res.results[2 * b]["y_out"] for b in range(Bn)])
    return y.astype(np.float32)
